# revision 11
# baseline (speedup 1.0000x reference)
"""3x3 valid conv (cross-correlation) + bias on a 4096x4096 fp32 image,
run across 8 trn2 NeuronCores.

Strategy
--------
Rows are sharded across the 8 cores host-side with a 2-row halo folded
into each core's input slice (no device collectives needed). On each
core the conv is computed as banded matmuls on the TensorEngine:

  For an output row-tile of M=126 rows (input rows K=M+2), and each of
  the 3 kernel columns dj, build a banded stationary matrix
  B_dj[k, m] = w[k-m, dj] (zero outside 0<=k-m<=2). Then

      Y_tile[m, n] = sum_dj sum_k B_dj[k, m] * X_tile[k, n+dj]

  i.e. matmuls accumulating in PSUM per 512-wide column chunk, with
  the dj shift expressed in the rhs access pattern. Bias is added during
  the PSUM->SBUF copy on the VectorEngine (DMA cannot read PSUM).

Variants (VARIANT):
  f32  - exact fp32 matmuls (4 cycles/row on the PE: slowest)
  f32r - TF32-like fp32r matmuls (1 cycle/row, ~2e-4 rel err)
  f16  - host-casts X and the bands to fp16 (halves input DMA,
         1 cycle/row, ~1.5e-4 rel err)
  f16c - compensated fp16: X = Xh + Xl, W = Wh + Wl (both splits exact
         to ~2^-22), Y = Wh@Xh + Wl@Xh + Wh@Xl. 9 matmuls/chunk but
         ~5e-7 rel err at 1 cycle/row.

Input and output DMAs are split into column halves for pipeline overlap.
"""

import numpy as np

H = 4096
W = 4096
KH = 3
KW = 3
HOUT = H - KH + 1  # 4094
WOUT = W - KW + 1  # 4094
NCORES = 8
ROWS_PER_CORE = 512          # output rows computed per core
IN_ROWS = ROWS_PER_CORE + 2  # input rows per core (with halo)
# Core 7 overlaps core 6 by 2 rows so that all shards have equal shape.
STARTS = [0, 512, 1024, 1536, 2048, 2560, 3072, 3582]
M_TILE = 126                 # output rows per matmul tile (K = M + 2 <= 128)
N_CHUNK = 512                # PSUM bank = 512 fp32

# input column halves (half B starts early so chunk 4's dj-shifted reads fit)
XA_LO, XA_W = 0, 2052
XB_LO, XB_W = 2046, 2050
YSPLIT = 2048                # output column split

# remainder strip: last R_STRIP output rows, packed as NSEG column
# segments stacked across partitions (NSEG*STRIP_IN partitions)
N_FULL_TILES = ROWS_PER_CORE // M_TILE   # 4
R_STRIP = ROWS_PER_CORE - N_FULL_TILES * M_TILE  # 8
STRIP_IN = R_STRIP + 2                   # 10
NSEG = 8
SEG = W // NSEG                          # 512

VARIANT = "f16"

_PROGRAM_CACHE = {}


def _build_program_v2():
    """f16 variant, restructured:
    - one x tile [128, 4096] per 126-row output tile (tile 0 split into
      two column halves so the first matmuls start early)
    - 3 matmuls per 1024-col chunk pair writing a 2-bank PSUM tile
      (halves matmul/LDWEIGHTS count vs 512-col chunks)
    - PSUM->SBUF bias-add copies alternate Vector / Scalar(ACT) so no
      single engine serializes the pipeline (GpSimd has no PSUM port)
    - fp16 y, stores per tile half on the scalar HWDGE ring
    """
    import concourse.mybir as mybir
    from concourse import bacc
    from concourse.tile import TileContext

    f32 = mybir.dt.float32
    f16 = mybir.dt.float16

    nc = bacc.Bacc()
    x = nc.declare_dram_parameter("x0", [IN_ROWS, W], f16, isOutput=False)
    bands = nc.declare_dram_parameter("bands", [128, 2 * KW, 128], f16, isOutput=False)
    biasc = nc.declare_dram_parameter("biasc", [128, 1], f32, isOutput=False)
    s0 = nc.declare_dram_parameter(
        "s0", [NSEG * STRIP_IN, SEG + 2], f16, isOutput=False
    )
    y = nc.declare_dram_parameter("y", [ROWS_PER_CORE, WOUT], f16, isOutput=True)
    ysd = nc.declare_dram_parameter("ys", [NSEG * R_STRIP, SEG], f16, isOutput=True)

    PAIRS = [(0, 1024), (1024, 1024), (2048, 1024), (3072, WOUT - 3072)]
    YSPL = 2048

    with TileContext(nc) as tc:
        with (
            tc.tile_pool(name="consts", bufs=1) as consts,
            tc.tile_pool(name="xp", bufs=N_FULL_TILES) as xp,
            tc.tile_pool(name="yp", bufs=4) as yp,
            tc.tile_pool(name="pp", bufs=4, space="PSUM") as pp,
        ):
            bands_sb = consts.tile([128, 2 * KW, 128], f16)
            nc.sync.dma_start(out=bands_sb[:], in_=bands[:])
            bias_sb = consts.tile([128, 1], f32)
            nc.sync.dma_start(out=bias_sb[:], in_=biasc[:])

            for t in range(N_FULL_TILES):
                r0 = t * M_TILE
                m = M_TILE
                k = m + KH - 1
                xt = xp.tile([128, W], f16, tag="x")
                if t == 0:
                    # first pair only needs cols < 2050; split so the PE
                    # can start after half the bytes land
                    nc.sync.dma_start(out=xt[:k, :2050], in_=x[r0 : r0 + k, :2050])
                    nc.sync.dma_start(out=xt[:k, 2050:], in_=x[r0 : r0 + k, 2050:])
                else:
                    nc.sync.dma_start(out=xt[:k, :], in_=x[r0 : r0 + k, :])
                ya = yp.tile([128, YSPL], f16, tag="ya")
                yb = yp.tile([128, WOUT - YSPL], f16, tag="yb")
                for p, (n0, nw) in enumerate(PAIRS):
                    pt = pp.tile([128, 2, N_CHUNK], f32, tag="pt")
                    for dj in range(KW):
                        for j in range(2):
                            c0 = n0 + j * N_CHUNK
                            cn = min(N_CHUNK, n0 + nw - c0)
                            nc.tensor.matmul(
                                pt[:128, j, :cn],
                                bands_sb[:k, dj, :],
                                xt[:k, c0 + dj : c0 + dj + cn],
                                start=(dj == 0),
                                stop=(dj == KW - 1),
                            )
                    yt = ya if n0 < YSPL else yb
                    off = n0 - (0 if n0 < YSPL else YSPL)
                    use_v = p % 2 == 0
                    if nw == 2 * N_CHUNK:
                        dst3 = yt[:m, off : off + nw].rearrange(
                            "m (a b) -> m a b", b=N_CHUNK
                        )
                        if use_v:
                            nc.vector.tensor_scalar_add(
                                dst3, pt[:m, :, :], bias_sb[:m, :]
                            )
                        else:
                            nc.scalar.add(dst3, pt[:m, :, :], bias_sb[:m, :])
                    else:
                        for j in range(2):
                            c0 = j * N_CHUNK
                            cn = min(N_CHUNK, nw - c0)
                            if use_v:
                                nc.vector.tensor_scalar_add(
                                    yt[:m, off + c0 : off + c0 + cn],
                                    pt[:m, j, :cn],
                                    bias_sb[:m, :],
                                )
                            else:
                                nc.scalar.add(
                                    yt[:m, off + c0 : off + c0 + cn],
                                    pt[:m, j, :cn],
                                    bias_sb[:m, :],
                                )
                    if p == 1:
                        nc.scalar.dma_start(
                            out=y[r0 : r0 + m, :YSPL], in_=ya[:m, :]
                        )
                    elif p == 3:
                        nc.scalar.dma_start(
                            out=y[r0 : r0 + m, YSPL:], in_=yb[:m, :]
                        )

            # remainder strip: last R_STRIP output rows packed as NSEG
            # column segments stacked across partitions
            npart = NSEG * STRIP_IN  # 80
            nout = NSEG * R_STRIP  # 64
            st = xp.tile([npart, SEG + 2], f16, tag="strip")
            nc.sync.dma_start(out=st[:, :], in_=s0[:, :])
            ptS = pp.tile([128, 2, N_CHUNK], f32, tag="pt")
            for dj in range(KW):
                nc.tensor.matmul(
                    ptS[:128, 0, :SEG],
                    bands_sb[:npart, KW + dj, :],
                    st[:npart, dj : dj + SEG],
                    start=(dj == 0),
                    stop=(dj == KW - 1),
                )
            ys = yp.tile([nout, SEG], f16, tag="ystrip")
            nc.vector.tensor_scalar_add(
                ys[:, :], ptS[:nout, 0, :SEG], bias_sb[:nout, :]
            )
            nc.scalar.dma_start(out=ysd[:, :], in_=ys[:, :])
    nc.finalize()
    return nc


def _build_program(variant: str):
    if variant == "f16":
        return _build_program_v2()
    import concourse.mybir as mybir
    from concourse import bacc
    from concourse.tile import TileContext

    f32 = mybir.dt.float32
    mm_dt = {
        "f32": f32,
        "f32r": mybir.dt.float32r,
        "f16": mybir.dt.float16,
        "f16c": mybir.dt.float16,
    }[variant]
    # fp16 output (host casts back to fp32): |Y| <= ~23 so quantization is
    # ~8e-3 abs, far inside the 2e-2-of-max gate; halves the output DMA.
    out_dt = f32 if variant == "f32" else mybir.dt.float16
    nterms = 3 if variant == "f16c" else 1

    nc = bacc.Bacc()
    # x inputs: one per term-split (f16c needs hi and lo parts)
    nxparts = 2 if variant == "f16c" else 1
    xs = [
        nc.declare_dram_parameter(f"x{i}", [IN_ROWS, W], mm_dt, isOutput=False)
        for i in range(nxparts)
    ]
    # bands: full-tile group + strip group; for f16c nwparts=2 (hi, lo)
    nwparts = 2 if variant == "f16c" else 1
    bands = nc.declare_dram_parameter(
        "bands", [128, 2 * KW * nwparts, 128], mm_dt, isOutput=False
    )
    biasc = nc.declare_dram_parameter("biasc", [128, 1], f32, isOutput=False)
    # host-packed remainder strip: NSEG column segments of the last
    # STRIP_IN input rows stacked across partitions, with 2-col halos
    ss = [
        nc.declare_dram_parameter(
            f"s{i}", [NSEG * STRIP_IN, SEG + 2], mm_dt, isOutput=False
        )
        for i in range(nxparts)
    ]
    y = nc.declare_dram_parameter(
        "y", [ROWS_PER_CORE, WOUT], out_dt, isOutput=True
    )
    ysd = nc.declare_dram_parameter(
        "ys", [NSEG * R_STRIP, SEG], out_dt, isOutput=True
    )

    n_tiles = N_FULL_TILES
    chunks = []
    n0 = 0
    while n0 < WOUT:
        chunks.append((n0, min(N_CHUNK, WOUT - n0)))
        n0 += N_CHUNK

    # (weight part index, x part index) per accumulation term:
    # f16c: Wh@Xh + Wl@Xh + Wh@Xl
    terms = [(0, 0), (1, 0), (0, 1)] if variant == "f16c" else [(0, 0)]

    with TileContext(nc) as tc:
        with (
            tc.tile_pool(name="consts", bufs=1) as consts,
            tc.tile_pool(name="xp", bufs=n_tiles) as xp,
            tc.tile_pool(name="yp", bufs=4) as yp,
            tc.tile_pool(name="pp", bufs=4, space="PSUM") as pp,
        ):
            bands_sb = consts.tile([128, 2 * KW * nwparts, 128], mm_dt)
            nc.sync.dma_start(out=bands_sb[:], in_=bands[:])
            bias_sb = consts.tile([128, 1], f32)
            nc.sync.dma_start(out=bias_sb[:], in_=biasc[:])

            # x loads issue on the Sync sequencer; y stores issue on the
            # Scalar sequencer (also HWDGE) so store waits never head-of-line
            # block the input stream.
            for t in range(n_tiles):
                r0 = t * M_TILE
                m = min(M_TILE, ROWS_PER_CORE - r0)
                k = m + KH - 1
                # load input halves for every x part
                xa = []
                xb = []
                for i in range(nxparts):
                    ta = xp.tile([128, XA_W], mm_dt, tag=f"xa{i}")
                    nc.sync.dma_start(
                        out=ta[:k, :], in_=xs[i][r0 : r0 + k, XA_LO : XA_LO + XA_W]
                    )
                    xa.append(ta)
                    tb = xp.tile([128, XB_W], mm_dt, tag=f"xb{i}")
                    nc.sync.dma_start(
                        out=tb[:k, :], in_=xs[i][r0 : r0 + k, XB_LO : XB_LO + XB_W]
                    )
                    xb.append(tb)
                ya = yp.tile([128, YSPLIT], out_dt, tag="ya")
                yb = yp.tile([128, WOUT - YSPLIT], out_dt, tag="yb")
                # process chunks in pairs sharing a 2-bank PSUM tile; one DVE
                # bias-add copy per pair halves the DVE instruction count.
                # Matmuls go weight-major (all users of one stationary matrix
                # back to back) and use the full 128-wide band (garbage rows
                # >= m never leave PSUM) so FWL can kick in.
                nmm = len(terms) * KW
                for p in range(0, len(chunks), 2):
                    pair = chunks[p : p + 2]
                    pt = pp.tile([128, 2, N_CHUNK], f32, tag="pt")
                    first_half = pair[0][0] < YSPLIT
                    xt = xa if first_half else xb
                    base = XA_LO if first_half else XB_LO
                    ndone = [0, 0]
                    for dj in range(KW):
                        for wi in range(nwparts):
                            xis = [xi for wj, xi in terms if wj == wi]
                            lhsT = bands_sb[:k, wi * KW + dj, :]
                            for xi in xis:
                                for j, (n0, n) in enumerate(pair):
                                    rhs = xt[xi][
                                        :k, n0 - base + dj : n0 - base + dj + n
                                    ]
                                    nc.tensor.matmul(
                                        pt[:128, j, :n],
                                        lhsT,
                                        rhs,
                                        start=(ndone[j] == 0),
                                        stop=(ndone[j] == nmm - 1),
                                    )
                                    ndone[j] += 1
                    n0 = pair[0][0]
                    nw = sum(n for _, n in pair)
                    yt = ya if first_half else yb
                    yoff = n0 - (0 if first_half else YSPLIT)
                    if nw == 2 * N_CHUNK:
                        # one DVE op across both PSUM banks
                        dst3 = yt[:m, yoff : yoff + nw].rearrange(
                            "m (a b) -> m a b", b=N_CHUNK
                        )
                        nc.vector.tensor_scalar_add(
                            dst3, pt[:m, :, :], bias_sb[:m, :]
                        )
                    else:
                        off = yoff
                        for j, (_, nj) in enumerate(pair):
                            nc.vector.tensor_scalar_add(
                                yt[:m, off : off + nj],
                                pt[:m, j, :nj],
                                bias_sb[:m, :],
                            )
                            off += nj
                nc.scalar.dma_start(out=y[r0 : r0 + m, :YSPLIT], in_=ya[:m, :])
                nc.scalar.dma_start(out=y[r0 : r0 + m, YSPLIT:], in_=yb[:m, :])

            # remainder strip: rows [N_FULL_TILES*M_TILE, ROWS_PER_CORE) for
            # all columns, as NSEG partition-stacked column segments. One
            # 512-wide chunk computes the whole strip.
            npart = NSEG * STRIP_IN           # 80
            nout = NSEG * R_STRIP             # 64
            strips = []
            for i in range(nxparts):
                st = xp.tile([npart, SEG + 2], mm_dt, tag=f"strip{i}")
                nc.sync.dma_start(out=st[:, :], in_=ss[i][:, :])
                strips.append(st)
            ptS = pp.tile([128, 2, N_CHUNK], f32, tag="pt")
            nmm = len(terms) * KW
            ndone = 0
            for dj in range(KW):
                for wi in range(nwparts):
                    xis = [xi for wj, xi in terms if wj == wi]
                    lhsT = bands_sb[:npart, (nwparts + wi) * KW + dj, :]
                    for xi in xis:
                        rhs = strips[xi][:npart, dj : dj + SEG]
                        nc.tensor.matmul(
                            ptS[:128, 0, :SEG],
                            lhsT,
                            rhs,
                            start=(ndone == 0),
                            stop=(ndone == nmm - 1),
                        )
                        ndone += 1
            ys = yp.tile([nout, SEG], out_dt, tag="ystrip")
            nc.vector.tensor_scalar_add(
                ys[:, :], ptS[:nout, 0, :SEG], bias_sb[:nout, :]
            )
            nc.scalar.dma_start(out=ysd[:, :], in_=ys[:, :])
    nc.finalize()
    return nc


def _get_program(variant: str):
    if variant not in _PROGRAM_CACHE:
        _PROGRAM_CACHE[variant] = _build_program(variant)
    return _PROGRAM_CACHE[variant]


def _make_bands(w_parts) -> np.ndarray:
    """w_parts: list of [KH, KW] arrays (one per weight split part)."""
    nw = len(w_parts)
    dtype = w_parts[0].dtype
    # full 128-wide bands: columns >= M_TILE produce garbage output rows
    # that are never copied out of PSUM, but make NumWeights==128 (FWL).
    # Second group: block-diagonal bands for the segment-packed remainder
    # strip (NSEG column segments of the last R_STRIP output rows stacked
    # across partitions).
    bands = np.zeros((128, 2 * KW * nw, 128), dtype)
    for wi, wp in enumerate(w_parts):
        for dj in range(KW):
            for d in range(KH):
                idx = np.arange(128 - d)
                bands[idx + d, wi * KW + dj, idx] = wp[d, dj]
            for blk in range(NSEG):
                for rp in range(R_STRIP):
                    for d in range(KH):
                        bands[
                            STRIP_IN * blk + rp + d,
                            KW * nw + wi * KW + dj,
                            R_STRIP * blk + rp,
                        ] = wp[d, dj]
    return bands


def _run(X, weight, bias, trace=False, variant=None, tmpdir=None):
    from concourse.bass_utils import run_bass_kernel_spmd

    variant = variant or VARIANT
    X = np.ascontiguousarray(np.asarray(X, dtype=np.float32))
    w = np.asarray(weight, dtype=np.float32)
    b = np.asarray(bias, dtype=np.float32)
    assert X.shape == (H, W) and w.shape == (KH, KW)

    nc = _get_program(variant)

    if variant == "f16c":
        Xh = X.astype(np.float16)
        Xl = (X - Xh.astype(np.float32)).astype(np.float16)
        wh = w.astype(np.float16)
        wl = (w - wh.astype(np.float32)).astype(np.float16)
        bands = _make_bands([wh, wl])
        xparts = [Xh, Xl]
    elif variant == "f16":
        bands = _make_bands([w.astype(np.float16)])
        xparts = [X.astype(np.float16)]
    else:
        bands = _make_bands([w])
        xparts = [X]

    biasc = np.full((128, 1), b[0], np.float32)

    def pack_strip(xp_arr, s):
        rs = s + N_FULL_TILES * M_TILE
        strip = xp_arr[rs : rs + STRIP_IN]  # [10, 4096]
        packed = np.zeros((NSEG * STRIP_IN, SEG + 2), xp_arr.dtype)
        packed[:, :SEG] = (
            strip.reshape(STRIP_IN, NSEG, SEG).transpose(1, 0, 2).reshape(-1, SEG)
        )
        halo = (
            strip[:, SEG:]
            .reshape(STRIP_IN, NSEG - 1, SEG)
            .transpose(1, 0, 2)
            .reshape(-1, SEG)[:, :2]
        )
        packed[: (NSEG - 1) * STRIP_IN, SEG : SEG + 2] = halo
        return packed

    in_maps = []
    for s in STARTS:
        m = {f"x{i}": xp[s : s + IN_ROWS] for i, xp in enumerate(xparts)}
        for i, xp in enumerate(xparts):
            m[f"s{i}"] = pack_strip(xp, s)
        m["bands"] = bands
        m["biasc"] = biasc
        in_maps.append(m)
    res = run_bass_kernel_spmd(
        nc, in_maps, core_ids=list(range(NCORES)), trace=trace, tmpdir=tmpdir
    )

    def core_block(c):
        blk = np.empty((ROWS_PER_CORE, WOUT), np.float32)
        r = res.results[c]
        blk[: N_FULL_TILES * M_TILE] = r["y"][: N_FULL_TILES * M_TILE]
        ys = r["ys"]  # [NSEG*R_STRIP, SEG] packed strip output
        for b_ in range(NSEG):
            wdt = min(SEG, WOUT - b_ * SEG)
            blk[N_FULL_TILES * M_TILE :, b_ * SEG : b_ * SEG + wdt] = ys[
                b_ * R_STRIP : (b_ + 1) * R_STRIP, :wdt
            ]
        return blk

    out = np.empty((HOUT, WOUT), np.float32)
    for c in range(NCORES - 1):
        out[STARTS[c] : STARTS[c] + ROWS_PER_CORE] = core_block(c)
    out[STARTS[-1] + 2 :] = core_block(NCORES - 1)[2:]
    return out, res.exec_time_ns


def kernel(X, weight, bias):
    out, _ = _run(X, weight, bias, trace=False)
    return out



# revision 14
# speedup vs baseline: 1.0692x; 1.0692x over previous
"""3x3 valid conv (cross-correlation) + bias on a 4096x4096 fp32 image,
run across 8 trn2 NeuronCores.

Strategy
--------
Rows are sharded across the 8 cores host-side with a 2-row halo folded
into each core's input slice (no device collectives needed). On each
core the conv is computed as banded matmuls on the TensorEngine:

  For an output row-tile of M=126 rows (input rows K=M+2), and each of
  the 3 kernel columns dj, build a banded stationary matrix
  B_dj[k, m] = w[k-m, dj] (zero outside 0<=k-m<=2). Then

      Y_tile[m, n] = sum_dj sum_k B_dj[k, m] * X_tile[k, n+dj]

  i.e. matmuls accumulating in PSUM per 512-wide column chunk, with
  the dj shift expressed in the rhs access pattern. Bias is added during
  the PSUM->SBUF copy on the VectorEngine (DMA cannot read PSUM).

Variants (VARIANT):
  f32  - exact fp32 matmuls (4 cycles/row on the PE: slowest)
  f32r - TF32-like fp32r matmuls (1 cycle/row, ~2e-4 rel err)
  f16  - host-casts X and the bands to fp16 (halves input DMA,
         1 cycle/row, ~1.5e-4 rel err)
  f16c - compensated fp16: X = Xh + Xl, W = Wh + Wl (both splits exact
         to ~2^-22), Y = Wh@Xh + Wl@Xh + Wh@Xl. 9 matmuls/chunk but
         ~5e-7 rel err at 1 cycle/row.

Input and output DMAs are split into column halves for pipeline overlap.
"""

import numpy as np

H = 4096
W = 4096
KH = 3
KW = 3
HOUT = H - KH + 1  # 4094
WOUT = W - KW + 1  # 4094
NCORES = 8
ROWS_PER_CORE = 512          # output rows computed per core
IN_ROWS = ROWS_PER_CORE + 2  # input rows per core (with halo)
# Core 7 overlaps core 6 by 2 rows so that all shards have equal shape.
STARTS = [0, 512, 1024, 1536, 2048, 2560, 3072, 3582]
M_TILE = 126                 # output rows per matmul tile (K = M + 2 <= 128)
N_CHUNK = 512                # PSUM bank = 512 fp32

# input column halves (half B starts early so chunk 4's dj-shifted reads fit)
XA_LO, XA_W = 0, 2052
XB_LO, XB_W = 2046, 2050
YSPLIT = 2048                # output column split

# remainder strip: last R_STRIP output rows, packed as NSEG column
# segments stacked across partitions (NSEG*STRIP_IN partitions)
N_FULL_TILES = ROWS_PER_CORE // M_TILE   # 4
R_STRIP = ROWS_PER_CORE - N_FULL_TILES * M_TILE  # 8
STRIP_IN = R_STRIP + 2                   # 10
NSEG = 8
SEG = W // NSEG                          # 512

VARIANT = "f16"

_PROGRAM_CACHE = {}


def _build_program_v2():
    """f16 variant, restructured:
    - one x tile [128, 4096] per 126-row output tile (tile 0 split into
      two column halves so the first matmuls start early)
    - 3 matmuls per 1024-col chunk pair writing a 2-bank PSUM tile
      (halves matmul/LDWEIGHTS count vs 512-col chunks)
    - PSUM->SBUF bias-add copies alternate Vector / Scalar(ACT) so no
      single engine serializes the pipeline (GpSimd has no PSUM port)
    - fp16 y, stores per tile half on the scalar HWDGE ring
    """
    import concourse.mybir as mybir
    from concourse import bacc
    from concourse.tile import TileContext

    f32 = mybir.dt.float32
    f16 = mybir.dt.float16

    nc = bacc.Bacc()
    x = nc.declare_dram_parameter("x0", [IN_ROWS, W], f16, isOutput=False)
    bands = nc.declare_dram_parameter("bands", [128, 2 * KW, 128], f16, isOutput=False)
    biasc = nc.declare_dram_parameter("biasc", [128, 1], f32, isOutput=False)
    s0 = nc.declare_dram_parameter(
        "s0", [NSEG * STRIP_IN, SEG + 2], f16, isOutput=False
    )
    y = nc.declare_dram_parameter("y", [ROWS_PER_CORE, WOUT], f16, isOutput=True)
    ysd = nc.declare_dram_parameter("ys", [NSEG * R_STRIP, SEG], f16, isOutput=True)

    PAIRS = [(0, 1024), (1024, 1024), (2048, 1024), (3072, WOUT - 3072)]
    YSPL = 2048

    with TileContext(nc) as tc:
        with (
            tc.tile_pool(name="consts", bufs=1) as consts,
            tc.tile_pool(name="xp", bufs=N_FULL_TILES) as xp,
            tc.tile_pool(name="yp", bufs=4) as yp,
            tc.tile_pool(name="pp", bufs=4, space="PSUM") as pp,
        ):
            bands_sb = consts.tile([128, 2 * KW, 128], f16)
            nc.sync.dma_start(out=bands_sb[:], in_=bands[:])
            bias_sb = consts.tile([128, 1], f32)
            nc.sync.dma_start(out=bias_sb[:], in_=biasc[:])

            # Warm the PE HAM clock gate during the initial load window:
            # ~8 dummy matmuls on a zeroed tile keep the PE busy >3.4us so
            # the real matmuls start at 2.4GHz instead of 1.2GHz.
            wt = consts.tile([128, 640], f16)
            nc.vector.memset(wt[:], 0.0)
            ptW = pp.tile([128, 2, N_CHUNK], f32, tag="pt")
            for i in range(8):
                nc.tensor.matmul(
                    ptW[:128, 0, :N_CHUNK],
                    wt[:128, :128],
                    wt[:128, 128:640],
                    start=(i == 0),
                    stop=(i == 7),
                )

            for t in range(N_FULL_TILES):
                r0 = t * M_TILE
                m = M_TILE
                k = m + KH - 1
                xt = xp.tile([128, W], f16, tag="x")
                if t == 0:
                    # graded loads: the first matmul only needs cols 0..513,
                    # so stage tile 0 in four pieces
                    for c0, c1 in ((0, 516), (516, 1540), (1540, 2564), (2564, W)):
                        nc.sync.dma_start(
                            out=xt[:k, c0:c1], in_=x[r0 : r0 + k, c0:c1]
                        )
                else:
                    nc.sync.dma_start(out=xt[:k, :], in_=x[r0 : r0 + k, :])
                ya = yp.tile([128, YSPL], f16, tag="ya")
                yb = yp.tile([128, WOUT - YSPL], f16, tag="yb")
                for p, (n0, nw) in enumerate(PAIRS):
                    pt = pp.tile([128, 2, N_CHUNK], f32, tag="pt")
                    for dj in range(KW):
                        for j in range(2):
                            c0 = n0 + j * N_CHUNK
                            cn = min(N_CHUNK, n0 + nw - c0)
                            nc.tensor.matmul(
                                pt[:128, j, :cn],
                                bands_sb[:k, dj, :],
                                xt[:k, c0 + dj : c0 + dj + cn],
                                start=(dj == 0),
                                stop=(dj == KW - 1),
                            )
                    yt = ya if n0 < YSPL else yb
                    off = n0 - (0 if n0 < YSPL else YSPL)
                    use_v = p % 2 == 0
                    if nw == 2 * N_CHUNK:
                        dst3 = yt[:m, off : off + nw].rearrange(
                            "m (a b) -> m a b", b=N_CHUNK
                        )
                        if use_v:
                            nc.vector.tensor_scalar_add(
                                dst3, pt[:m, :, :], bias_sb[:m, :]
                            )
                        else:
                            nc.scalar.add(dst3, pt[:m, :, :], bias_sb[:m, :])
                    else:
                        for j in range(2):
                            c0 = j * N_CHUNK
                            cn = min(N_CHUNK, nw - c0)
                            if use_v:
                                nc.vector.tensor_scalar_add(
                                    yt[:m, off + c0 : off + c0 + cn],
                                    pt[:m, j, :cn],
                                    bias_sb[:m, :],
                                )
                            else:
                                nc.scalar.add(
                                    yt[:m, off + c0 : off + c0 + cn],
                                    pt[:m, j, :cn],
                                    bias_sb[:m, :],
                                )
                    if t == N_FULL_TILES - 1:
                        # last tile: loads are long done, so the sync HWDGE
                        # ring is free; store per pair to shrink the tail
                        nc.sync.dma_start(
                            out=y[r0 : r0 + m, n0 : n0 + nw],
                            in_=yt[:m, off : off + nw],
                        )
                    elif p == 1:
                        nc.scalar.dma_start(
                            out=y[r0 : r0 + m, :YSPL], in_=ya[:m, :]
                        )
                    elif p == 3:
                        nc.scalar.dma_start(
                            out=y[r0 : r0 + m, YSPL:], in_=yb[:m, :]
                        )

            # remainder strip: last R_STRIP output rows packed as NSEG
            # column segments stacked across partitions
            npart = NSEG * STRIP_IN  # 80
            nout = NSEG * R_STRIP  # 64
            st = xp.tile([npart, SEG + 2], f16, tag="strip")
            nc.sync.dma_start(out=st[:, :], in_=s0[:, :])
            ptS = pp.tile([128, 2, N_CHUNK], f32, tag="pt")
            for dj in range(KW):
                nc.tensor.matmul(
                    ptS[:128, 0, :SEG],
                    bands_sb[:npart, KW + dj, :],
                    st[:npart, dj : dj + SEG],
                    start=(dj == 0),
                    stop=(dj == KW - 1),
                )
            ys = yp.tile([nout, SEG], f16, tag="ystrip")
            nc.vector.tensor_scalar_add(
                ys[:, :], ptS[:nout, 0, :SEG], bias_sb[:nout, :]
            )
            nc.sync.dma_start(out=ysd[:, :], in_=ys[:, :])
    nc.finalize()
    return nc


def _build_program(variant: str):
    if variant == "f16":
        return _build_program_v2()
    import concourse.mybir as mybir
    from concourse import bacc
    from concourse.tile import TileContext

    f32 = mybir.dt.float32
    mm_dt = {
        "f32": f32,
        "f32r": mybir.dt.float32r,
        "f16": mybir.dt.float16,
        "f16c": mybir.dt.float16,
    }[variant]
    # fp16 output (host casts back to fp32): |Y| <= ~23 so quantization is
    # ~8e-3 abs, far inside the 2e-2-of-max gate; halves the output DMA.
    out_dt = f32 if variant == "f32" else mybir.dt.float16
    nterms = 3 if variant == "f16c" else 1

    nc = bacc.Bacc()
    # x inputs: one per term-split (f16c needs hi and lo parts)
    nxparts = 2 if variant == "f16c" else 1
    xs = [
        nc.declare_dram_parameter(f"x{i}", [IN_ROWS, W], mm_dt, isOutput=False)
        for i in range(nxparts)
    ]
    # bands: full-tile group + strip group; for f16c nwparts=2 (hi, lo)
    nwparts = 2 if variant == "f16c" else 1
    bands = nc.declare_dram_parameter(
        "bands", [128, 2 * KW * nwparts, 128], mm_dt, isOutput=False
    )
    biasc = nc.declare_dram_parameter("biasc", [128, 1], f32, isOutput=False)
    # host-packed remainder strip: NSEG column segments of the last
    # STRIP_IN input rows stacked across partitions, with 2-col halos
    ss = [
        nc.declare_dram_parameter(
            f"s{i}", [NSEG * STRIP_IN, SEG + 2], mm_dt, isOutput=False
        )
        for i in range(nxparts)
    ]
    y = nc.declare_dram_parameter(
        "y", [ROWS_PER_CORE, WOUT], out_dt, isOutput=True
    )
    ysd = nc.declare_dram_parameter(
        "ys", [NSEG * R_STRIP, SEG], out_dt, isOutput=True
    )

    n_tiles = N_FULL_TILES
    chunks = []
    n0 = 0
    while n0 < WOUT:
        chunks.append((n0, min(N_CHUNK, WOUT - n0)))
        n0 += N_CHUNK

    # (weight part index, x part index) per accumulation term:
    # f16c: Wh@Xh + Wl@Xh + Wh@Xl
    terms = [(0, 0), (1, 0), (0, 1)] if variant == "f16c" else [(0, 0)]

    with TileContext(nc) as tc:
        with (
            tc.tile_pool(name="consts", bufs=1) as consts,
            tc.tile_pool(name="xp", bufs=n_tiles) as xp,
            tc.tile_pool(name="yp", bufs=4) as yp,
            tc.tile_pool(name="pp", bufs=4, space="PSUM") as pp,
        ):
            bands_sb = consts.tile([128, 2 * KW * nwparts, 128], mm_dt)
            nc.sync.dma_start(out=bands_sb[:], in_=bands[:])
            bias_sb = consts.tile([128, 1], f32)
            nc.sync.dma_start(out=bias_sb[:], in_=biasc[:])

            # x loads issue on the Sync sequencer; y stores issue on the
            # Scalar sequencer (also HWDGE) so store waits never head-of-line
            # block the input stream.
            for t in range(n_tiles):
                r0 = t * M_TILE
                m = min(M_TILE, ROWS_PER_CORE - r0)
                k = m + KH - 1
                # load input halves for every x part
                xa = []
                xb = []
                for i in range(nxparts):
                    ta = xp.tile([128, XA_W], mm_dt, tag=f"xa{i}")
                    nc.sync.dma_start(
                        out=ta[:k, :], in_=xs[i][r0 : r0 + k, XA_LO : XA_LO + XA_W]
                    )
                    xa.append(ta)
                    tb = xp.tile([128, XB_W], mm_dt, tag=f"xb{i}")
                    nc.sync.dma_start(
                        out=tb[:k, :], in_=xs[i][r0 : r0 + k, XB_LO : XB_LO + XB_W]
                    )
                    xb.append(tb)
                ya = yp.tile([128, YSPLIT], out_dt, tag="ya")
                yb = yp.tile([128, WOUT - YSPLIT], out_dt, tag="yb")
                # process chunks in pairs sharing a 2-bank PSUM tile; one DVE
                # bias-add copy per pair halves the DVE instruction count.
                # Matmuls go weight-major (all users of one stationary matrix
                # back to back) and use the full 128-wide band (garbage rows
                # >= m never leave PSUM) so FWL can kick in.
                nmm = len(terms) * KW
                for p in range(0, len(chunks), 2):
                    pair = chunks[p : p + 2]
                    pt = pp.tile([128, 2, N_CHUNK], f32, tag="pt")
                    first_half = pair[0][0] < YSPLIT
                    xt = xa if first_half else xb
                    base = XA_LO if first_half else XB_LO
                    ndone = [0, 0]
                    for dj in range(KW):
                        for wi in range(nwparts):
                            xis = [xi for wj, xi in terms if wj == wi]
                            lhsT = bands_sb[:k, wi * KW + dj, :]
                            for xi in xis:
                                for j, (n0, n) in enumerate(pair):
                                    rhs = xt[xi][
                                        :k, n0 - base + dj : n0 - base + dj + n
                                    ]
                                    nc.tensor.matmul(
                                        pt[:128, j, :n],
                                        lhsT,
                                        rhs,
                                        start=(ndone[j] == 0),
                                        stop=(ndone[j] == nmm - 1),
                                    )
                                    ndone[j] += 1
                    n0 = pair[0][0]
                    nw = sum(n for _, n in pair)
                    yt = ya if first_half else yb
                    yoff = n0 - (0 if first_half else YSPLIT)
                    if nw == 2 * N_CHUNK:
                        # one DVE op across both PSUM banks
                        dst3 = yt[:m, yoff : yoff + nw].rearrange(
                            "m (a b) -> m a b", b=N_CHUNK
                        )
                        nc.vector.tensor_scalar_add(
                            dst3, pt[:m, :, :], bias_sb[:m, :]
                        )
                    else:
                        off = yoff
                        for j, (_, nj) in enumerate(pair):
                            nc.vector.tensor_scalar_add(
                                yt[:m, off : off + nj],
                                pt[:m, j, :nj],
                                bias_sb[:m, :],
                            )
                            off += nj
                nc.scalar.dma_start(out=y[r0 : r0 + m, :YSPLIT], in_=ya[:m, :])
                nc.scalar.dma_start(out=y[r0 : r0 + m, YSPLIT:], in_=yb[:m, :])

            # remainder strip: rows [N_FULL_TILES*M_TILE, ROWS_PER_CORE) for
            # all columns, as NSEG partition-stacked column segments. One
            # 512-wide chunk computes the whole strip.
            npart = NSEG * STRIP_IN           # 80
            nout = NSEG * R_STRIP             # 64
            strips = []
            for i in range(nxparts):
                st = xp.tile([npart, SEG + 2], mm_dt, tag=f"strip{i}")
                nc.sync.dma_start(out=st[:, :], in_=ss[i][:, :])
                strips.append(st)
            ptS = pp.tile([128, 2, N_CHUNK], f32, tag="pt")
            nmm = len(terms) * KW
            ndone = 0
            for dj in range(KW):
                for wi in range(nwparts):
                    xis = [xi for wj, xi in terms if wj == wi]
                    lhsT = bands_sb[:npart, (nwparts + wi) * KW + dj, :]
                    for xi in xis:
                        rhs = strips[xi][:npart, dj : dj + SEG]
                        nc.tensor.matmul(
                            ptS[:128, 0, :SEG],
                            lhsT,
                            rhs,
                            start=(ndone == 0),
                            stop=(ndone == nmm - 1),
                        )
                        ndone += 1
            ys = yp.tile([nout, SEG], out_dt, tag="ystrip")
            nc.vector.tensor_scalar_add(
                ys[:, :], ptS[:nout, 0, :SEG], bias_sb[:nout, :]
            )
            nc.scalar.dma_start(out=ysd[:, :], in_=ys[:, :])
    nc.finalize()
    return nc


def _get_program(variant: str):
    if variant not in _PROGRAM_CACHE:
        _PROGRAM_CACHE[variant] = _build_program(variant)
    return _PROGRAM_CACHE[variant]


def _make_bands(w_parts) -> np.ndarray:
    """w_parts: list of [KH, KW] arrays (one per weight split part)."""
    nw = len(w_parts)
    dtype = w_parts[0].dtype
    # full 128-wide bands: columns >= M_TILE produce garbage output rows
    # that are never copied out of PSUM, but make NumWeights==128 (FWL).
    # Second group: block-diagonal bands for the segment-packed remainder
    # strip (NSEG column segments of the last R_STRIP output rows stacked
    # across partitions).
    bands = np.zeros((128, 2 * KW * nw, 128), dtype)
    for wi, wp in enumerate(w_parts):
        for dj in range(KW):
            for d in range(KH):
                idx = np.arange(128 - d)
                bands[idx + d, wi * KW + dj, idx] = wp[d, dj]
            for blk in range(NSEG):
                for rp in range(R_STRIP):
                    for d in range(KH):
                        bands[
                            STRIP_IN * blk + rp + d,
                            KW * nw + wi * KW + dj,
                            R_STRIP * blk + rp,
                        ] = wp[d, dj]
    return bands


def _run(X, weight, bias, trace=False, variant=None, tmpdir=None):
    from concourse.bass_utils import run_bass_kernel_spmd

    variant = variant or VARIANT
    X = np.ascontiguousarray(np.asarray(X, dtype=np.float32))
    w = np.asarray(weight, dtype=np.float32)
    b = np.asarray(bias, dtype=np.float32)
    assert X.shape == (H, W) and w.shape == (KH, KW)

    nc = _get_program(variant)

    if variant == "f16c":
        Xh = X.astype(np.float16)
        Xl = (X - Xh.astype(np.float32)).astype(np.float16)
        wh = w.astype(np.float16)
        wl = (w - wh.astype(np.float32)).astype(np.float16)
        bands = _make_bands([wh, wl])
        xparts = [Xh, Xl]
    elif variant == "f16":
        bands = _make_bands([w.astype(np.float16)])
        xparts = [X.astype(np.float16)]
    else:
        bands = _make_bands([w])
        xparts = [X]

    biasc = np.full((128, 1), b[0], np.float32)

    def pack_strip(xp_arr, s):
        rs = s + N_FULL_TILES * M_TILE
        strip = xp_arr[rs : rs + STRIP_IN]  # [10, 4096]
        packed = np.zeros((NSEG * STRIP_IN, SEG + 2), xp_arr.dtype)
        packed[:, :SEG] = (
            strip.reshape(STRIP_IN, NSEG, SEG).transpose(1, 0, 2).reshape(-1, SEG)
        )
        halo = (
            strip[:, SEG:]
            .reshape(STRIP_IN, NSEG - 1, SEG)
            .transpose(1, 0, 2)
            .reshape(-1, SEG)[:, :2]
        )
        packed[: (NSEG - 1) * STRIP_IN, SEG : SEG + 2] = halo
        return packed

    in_maps = []
    for s in STARTS:
        m = {f"x{i}": xp[s : s + IN_ROWS] for i, xp in enumerate(xparts)}
        for i, xp in enumerate(xparts):
            m[f"s{i}"] = pack_strip(xp, s)
        m["bands"] = bands
        m["biasc"] = biasc
        in_maps.append(m)
    res = run_bass_kernel_spmd(
        nc, in_maps, core_ids=list(range(NCORES)), trace=trace, tmpdir=tmpdir
    )

    def core_block(c):
        blk = np.empty((ROWS_PER_CORE, WOUT), np.float32)
        r = res.results[c]
        blk[: N_FULL_TILES * M_TILE] = r["y"][: N_FULL_TILES * M_TILE]
        ys = r["ys"]  # [NSEG*R_STRIP, SEG] packed strip output
        for b_ in range(NSEG):
            wdt = min(SEG, WOUT - b_ * SEG)
            blk[N_FULL_TILES * M_TILE :, b_ * SEG : b_ * SEG + wdt] = ys[
                b_ * R_STRIP : (b_ + 1) * R_STRIP, :wdt
            ]
        return blk

    out = np.empty((HOUT, WOUT), np.float32)
    for c in range(NCORES - 1):
        out[STARTS[c] : STARTS[c] + ROWS_PER_CORE] = core_block(c)
    out[STARTS[-1] + 2 :] = core_block(NCORES - 1)[2:]
    return out, res.exec_time_ns


def kernel(X, weight, bias):
    out, _ = _run(X, weight, bias, trace=False)
    return out

